# revision 13
# baseline (speedup 1.0000x reference)
"""ChiENN message-passing attention kernel for 8 Trainium2 NeuronCores.

Reference computation (per node n, D=256, H=8 heads, hd=32, K=18 slots):
    all_msg = [ccw_msg(8), self_msg, parallel_msg, cw_msg(8)]   (N, 18, 256)
    q = all_msg @ Wq.T ; k = all_msg[:,0] @ Wk.T ; v = all_msg @ Wv.T
    scores = einsum('nkhd,nhd->nhk', q, k) / sqrt(32)  masked-softmax over k
    out = (einsum('nhk,nkhd->nhd', p, v) @ Wfinal.T)

Sharding: data-parallel over nodes, 3750 nodes/core (padded to 3840 = 30
tiles of 128).  parallel_node_index gather on host; W_self / W_parallel
folded into Wq / Wv on host.

Device layout per 128-node tile (node-major, partitions = nodes):
  matmuls:  lhsT = msgT [din_half, node], rhs = [Wq.T | Wv_perm.T]
            -> PSUM [node, q(256,(h,j)) | v(256,(j,h))] per slot
  scores:   DVE mult (PSUM q fp32) x (k bf16 bcast) -> prods bf16;
            j-reduce via binary tree of bf16 TensorTensor adds (2x mode)
  softmax:  +mask bias, ScalarE exp on 144 vals, denominator reduce,
            reciprocal folded into pexp144 BEFORE mixing (no post-scale)
  mixing:   v stored j-major [n, j, s, h] (Wv columns permuted on host) so
            the pexp broadcast is on the middle dim -> 2x DVE mult with NO
            expansion; s-reduce via bf16 tree (stage 1 on GpSimd, rest DVE)
  final:    PE transpose of mixed [n,(j,h)] -> lhsT, matmul with
            row-permuted Wfinal.
"""

import numpy as np
import ml_dtypes

N_TOTAL = 30000
D = 256
H = 8
HD = 32
L = 8
NSLOT = 18  # 8 ccw + self + parallel + 8 cw
NMSG = 16  # slots that come straight from ccw/cw inputs
NCORES = 8
NPC = N_TOTAL // NCORES  # 3750
NPAD = 3840  # 30 tiles of 128
TILES = NPAD // 128
BIG_NEG = -1.0e6  # additive pre-scale mask bias; exp((x+BIG_NEG)/sqrt32) == 0
INV_SQRT_HD = 1.0 / np.sqrt(32.0)

BF16 = ml_dtypes.bfloat16

_CACHE = {}


# --------------------------------------------------------------------------
# Device program
# --------------------------------------------------------------------------

def _build_program(tiles):
    import concourse.bass as bass
    import concourse.tile as tile
    from concourse import bacc, masks, mybir
    from contextlib import ExitStack

    dt = mybir.dt
    nc = bacc.Bacc("TRN2", target_bir_lowering=False, debug=False)

    msgs_d = nc.dram_tensor("msgs", [2, tiles, 128, NMSG * 128], dt.bfloat16,
                            kind="ExternalInput").ap()
    bx_d = nc.dram_tensor("bx", [2, tiles, 128, 128], dt.bfloat16,
                          kind="ExternalInput").ap()
    px_d = nc.dram_tensor("px", [2, tiles, 128, 128], dt.bfloat16,
                          kind="ExternalInput").ap()
    bias_d = nc.dram_tensor("bias", [tiles, 128, NSLOT], dt.float32,
                            kind="ExternalInput").ap()
    wmsg_d = nc.dram_tensor("wmsg", [2, 128, 512], dt.bfloat16,
                            kind="ExternalInput").ap()
    wself_d = nc.dram_tensor("wself", [2, 128, 512], dt.bfloat16,
                             kind="ExternalInput").ap()
    wpar_d = nc.dram_tensor("wpar", [2, 128, 512], dt.bfloat16,
                            kind="ExternalInput").ap()
    wk_d = nc.dram_tensor("wk", [2, 128, 256], dt.bfloat16,
                          kind="ExternalInput").ap()
    wfin_d = nc.dram_tensor("wfin", [2, 128, 256], dt.bfloat16,
                            kind="ExternalInput").ap()
    out_d = nc.dram_tensor("out", [tiles, 128, 256], dt.bfloat16,
                           kind="ExternalOutput").ap()

    with tile.TileContext(nc) as tc, ExitStack() as ctx:
        # ---- static tiles: weights + identity -------------------------------
        wpool = ctx.enter_context(tc.tile_pool(name="w", bufs=1))
        wmsg = [wpool.tile([128, 512], dt.bfloat16, tag=f"wmsg{i}", name=f"wmsg{i}") for i in range(2)]
        wself = [wpool.tile([128, 512], dt.bfloat16, tag=f"wself{i}", name=f"wself{i}") for i in range(2)]
        wpar = [wpool.tile([128, 512], dt.bfloat16, tag=f"wpar{i}", name=f"wpar{i}") for i in range(2)]
        wk = [wpool.tile([128, 256], dt.bfloat16, tag=f"wk{i}", name=f"wk{i}") for i in range(2)]
        wfin = [wpool.tile([128, 256], dt.bfloat16, tag=f"wfin{i}", name=f"wfin{i}") for i in range(2)]
        ident = wpool.tile([128, 128], dt.bfloat16, tag="ident")
        for i in range(2):
            nc.sync.dma_start(wmsg[i][:], wmsg_d[i])
            nc.sync.dma_start(wself[i][:], wself_d[i])
            nc.sync.dma_start(wpar[i][:], wpar_d[i])
            nc.sync.dma_start(wk[i][:], wk_d[i])
            nc.sync.dma_start(wfin[i][:], wfin_d[i])
        masks.make_identity(nc, ident[:])

        # ---- per-tile pools -------------------------------------------------
        msgp = ctx.enter_context(tc.tile_pool(name="msgs", bufs=8))
        xp = ctx.enter_context(tc.tile_pool(name="xs", bufs=8))
        biasp = ctx.enter_context(tc.tile_pool(name="bias", bufs=3))
        kp = ctx.enter_context(tc.tile_pool(name="ksb", bufs=3))
        prodp = ctx.enter_context(tc.tile_pool(name="prods", bufs=2))
        prod2p = ctx.enter_context(tc.tile_pool(name="prod2", bufs=3))
        vp = ctx.enter_context(tc.tile_pool(name="vsb", bufs=2))
        treep = ctx.enter_context(tc.tile_pool(name="tree", bufs=2))
        smallp = ctx.enter_context(tc.tile_pool(name="small", bufs=3))
        outp = ctx.enter_context(tc.tile_pool(name="outs", bufs=4))
        qvpsum = ctx.enter_context(
            tc.tile_pool(name="qvps", bufs=3, space="PSUM"))
        miscpsum = ctx.enter_context(
            tc.tile_pool(name="miscps", bufs=2, space="PSUM"))

        pending = []

        def emit_tail():
            tt, mixed = pending.pop(0)
            otp = miscpsum.tile([128, 256], dt.bfloat16, tag="misc")
            for half in range(2):
                nc.tensor.transpose(otp[:, half * 128:(half + 1) * 128],
                                    mixed[:, half * 128:(half + 1) * 128],
                                    ident[:])
            outT = outp.tile([128, 256], dt.bfloat16, tag="outT")
            nc.scalar.copy(outT[:], otp[:])
            fin = miscpsum.tile([128, 256], dt.float32, tag="misc")
            for half in range(2):
                nc.tensor.matmul(fin[:], outT[:, half * 128:(half + 1) * 128],
                                 wfin[half][:],
                                 start=(half == 0), stop=(half == 1))
            out_sb = outp.tile([128, 256], dt.bfloat16, tag="outsb")
            nc.scalar.copy(out_sb[:], fin[:])
            nc.sync.dma_start(out_d[tt], out_sb[:])

        for t in range(tiles):
            msg = [msgp.tile([128, NMSG * 128], dt.bfloat16, tag=f"msg{i}", name=f"msg{i}_{t}")
                   for i in range(2)]
            bxs = [xp.tile([128, 128], dt.bfloat16, tag=f"bx{i}", name=f"bx{i}_{t}") for i in range(2)]
            pxs = [xp.tile([128, 128], dt.bfloat16, tag=f"px{i}", name=f"px{i}_{t}") for i in range(2)]
            bias_sb = biasp.tile([128, NSLOT], dt.float32, tag="bias")
            for i in range(2):
                nc.sync.dma_start(msg[i][:], msgs_d[i, t])
                nc.sync.dma_start(bxs[i][:], bx_d[i, t])
                nc.sync.dma_start(pxs[i][:], px_d[i, t])
            nc.sync.dma_start(bias_sb[:], bias_d[t])

            def lhs(ls, dh):
                if ls == 8:
                    return bxs[dh][:]
                if ls == 9:
                    return pxs[dh][:]
                ms = ls if ls < 8 else ls - 2
                return msg[dh][:, ms * 128:(ms + 1) * 128]

            def rhs(ls, dh):
                if ls == 8:
                    return wself[dh][:]
                if ls == 9:
                    return wpar[dh][:]
                return wmsg[dh][:]

            k_sb = kp.tile([128, 256], dt.bfloat16, tag="ksb")
            prods = prodp.tile([128, NSLOT * 256], dt.bfloat16, tag="prods")
            # fused q|v SBUF stage: [n, s, 512] (q cols (h,j), v cols (j,h))
            qv_sb = vp.tile([128, NSLOT * 512], dt.bfloat16, tag="qvsb")
            qvs3 = qv_sb[:].rearrange("p (s c) -> p s c", s=NSLOT)

            for g in range(9):
                qv = qvpsum.tile([128, 2 * 512], dt.float32, tag="qv")
                for i in range(2):
                    ls = 2 * g + i
                    for dh in range(2):
                        nc.tensor.matmul(
                            qv[:, i * 512:(i + 1) * 512],
                            lhs(ls, dh), rhs(ls, dh),
                            start=(dh == 0), stop=(dh == 1))
                if g == 0:
                    kps = miscpsum.tile([128, 256], dt.float32, tag="misc")
                    for dh in range(2):
                        nc.tensor.matmul(kps[:], msg[dh][:, 0:128], wk[dh][:],
                                         start=(dh == 0), stop=(dh == 1))
                    nc.scalar.copy(k_sb[:], kps[:])

                qv3 = qv[:].rearrange("p (s x) -> p s x", s=2)
                k_bc = k_sb[:].unsqueeze(1).broadcast_to([128, 2, 256])
                pr3 = prods[:].rearrange("p (s c) -> p s c", s=NSLOT)
                # drain q first so the q*k mult starts immediately; v second
                nc.scalar.copy(qvs3[:, 2 * g:2 * g + 2, 0:256],
                               qv3[:, :, 0:256])
                nc.vector.tensor_mul(pr3[:, 2 * g:2 * g + 2, :],
                                     qvs3[:, 2 * g:2 * g + 2, 0:256], k_bc)
                nc.scalar.copy(qvs3[:, 2 * g:2 * g + 2, 256:512],
                               qv3[:, :, 256:512])

            if pending and t > 0:
                emit_tail()

            # ---- scores: j-reduce via bf16 binary tree (2x DVE) ------------
            pr4 = prods[:].rearrange("p (s h j) -> p s h j", s=NSLOT, h=H)
            t1 = treep.tile([128, NSLOT * H * 16], dt.bfloat16, tag="t1")
            t1v = t1[:].rearrange("p (s h j) -> p s h j", s=NSLOT, h=H)
            nc.vector.tensor_add(t1v, pr4[:, :, :, 0:16], pr4[:, :, :, 16:32])
            t2 = treep.tile([128, NSLOT * H * 8], dt.bfloat16, tag="t2")
            t2v = t2[:].rearrange("p (s h j) -> p s h j", s=NSLOT, h=H)
            nc.vector.tensor_add(t2v, t1v[:, :, :, 0:8], t1v[:, :, :, 8:16])
            t3 = treep.tile([128, NSLOT * H * 4], dt.bfloat16, tag="t3")
            t3v = t3[:].rearrange("p (s h j) -> p s h j", s=NSLOT, h=H)
            nc.vector.tensor_add(t3v, t2v[:, :, :, 0:4], t2v[:, :, :, 4:8])
            t4 = treep.tile([128, NSLOT * H * 2], dt.bfloat16, tag="t4")
            t4v = t4[:].rearrange("p (s h j) -> p s h j", s=NSLOT, h=H)
            nc.vector.tensor_add(t4v, t3v[:, :, :, 0:2], t3v[:, :, :, 2:4])
            # last stage + bias add in fp32
            scob = smallp.tile([128, NSLOT * H], dt.float32, tag="scob")
            scobv = scob[:].rearrange("p (s h) -> p s h", s=NSLOT)
            nc.vector.tensor_add(scobv, t4v[:, :, :, 0], t4v[:, :, :, 1])
            nc.vector.tensor_add(
                scobv, scobv,
                bias_sb[:].unsqueeze(2).broadcast_to([128, NSLOT, H]))

            # ---- softmax (normalized pexp, 144 vals) -----------------------
            pexp144 = smallp.tile([128, NSLOT * H], dt.bfloat16, tag="pexp144")
            nc.scalar.activation(
                pexp144[:], scob[:],
                mybir.ActivationFunctionType.Exp,
                scale=float(INV_SQRT_HD))
            denom = smallp.tile([128, H], dt.float32, tag="denom")
            nc.vector.reduce_sum(
                denom[:],
                pexp144[:].rearrange("p (s h) -> p s h", s=NSLOT)
                    .transpose([0, 2, 1]),
                axis=mybir.AxisListType.X)
            recip = smallp.tile([128, H], dt.float32, tag="recip")
            nc.vector.reciprocal(recip[:], denom[:])
            pexpn = smallp.tile([128, NSLOT * H], dt.bfloat16, tag="pexpn")
            nc.vector.tensor_mul(
                pexpn[:].rearrange("p (s h) -> p s h", s=NSLOT),
                pexp144[:].rearrange("p (s h) -> p s h", s=NSLOT),
                recip[:].unsqueeze(1).broadcast_to([128, NSLOT, H]))

            # ---- mixing: prod2[n,s,j,h] = v * pexpn (2x, no expansion) -----
            prod2 = prod2p.tile([128, NSLOT * HD * H], dt.bfloat16, tag="prod2")
            p2v = prod2[:].rearrange("p (s j h) -> p s j h", s=NSLOT, j=HD)
            v4 = qvs3[:, :, 256:512].rearrange("p s (j h) -> p s j h", j=HD)
            pexpn_bc = pexpn[:].rearrange("p (s h) -> p s h", s=NSLOT) \
                .unsqueeze(2).broadcast_to([128, NSLOT, HD, H])
            nc.vector.tensor_mul(p2v, v4, pexpn_bc)

            # s-reduce tree on DVE, contiguous s-major blocks (m5 on GpSimd)
            m1 = treep.tile([128, 8 * 256], dt.bfloat16, tag="m1")
            m1v = m1[:].rearrange("p (s c) -> p s c", s=8)
            p2c = prod2[:].rearrange("p (s c) -> p s c", s=NSLOT)
            nc.vector.tensor_add(m1v, p2c[:, 0:8, :], p2c[:, 8:16, :])
            m5 = treep.tile([128, 256], dt.bfloat16, tag="m5")
            nc.gpsimd.tensor_add(m5[:], p2c[:, 16, :], p2c[:, 17, :])
            m2 = treep.tile([128, 4 * 256], dt.bfloat16, tag="m2")
            m2v = m2[:].rearrange("p (s c) -> p s c", s=4)
            nc.vector.tensor_add(m2v, m1v[:, 0:4, :], m1v[:, 4:8, :])
            m3 = treep.tile([128, 2 * 256], dt.bfloat16, tag="m3")
            m3v = m3[:].rearrange("p (s c) -> p s c", s=2)
            nc.vector.tensor_add(m3v, m2v[:, 0:2, :], m2v[:, 2:4, :])
            m4 = treep.tile([128, 256], dt.bfloat16, tag="m4")
            nc.vector.tensor_add(m4[:], m3v[:, 0, :], m3v[:, 1, :])
            mixed = outp.tile([128, 256], dt.bfloat16, tag="mixed")
            nc.vector.tensor_add(mixed[:], m4[:], m5[:])
            pending.append((t, mixed))

        while pending:
            emit_tail()

    nc.compile()
    return nc


# --------------------------------------------------------------------------
# Host-side packing
# --------------------------------------------------------------------------

# column/row permutation: position (j*8+h) <- original channel (h*32+j)
_PERM_JH = np.arange(256).reshape(H, HD).T.reshape(-1)  # [j*8+h] = h*32+j


def _pack_weights(W_self, W_parallel, W_q, W_k, W_v, W_final):
    f32 = np.float32
    W_self = np.asarray(W_self, f32)
    W_parallel = np.asarray(W_parallel, f32)
    W_q = np.asarray(W_q, f32)
    W_k = np.asarray(W_k, f32)
    W_v = np.asarray(W_v, f32)
    W_final = np.asarray(W_final, f32)

    def halves(mat):  # (256, out) -> (2, 128, out) bf16
        return np.ascontiguousarray(
            mat.reshape(2, 128, mat.shape[1])).astype(BF16)

    wq_t = W_q.T  # (din, dout) dout cols (h,j)-packed
    wv_t = W_v.T[:, _PERM_JH]  # v cols (j,h)-packed
    wmsg = halves(np.concatenate([wq_t, wv_t], axis=1))
    wself = halves(np.concatenate(
        [(W_q @ W_self).T, (W_v @ W_self).T[:, _PERM_JH]], axis=1))
    wpar = halves(np.concatenate(
        [(W_q @ W_parallel).T, (W_v @ W_parallel).T[:, _PERM_JH]], axis=1))
    wk = halves(W_k.T)
    wfin = halves(W_final.T[_PERM_JH, :])  # rows (j,h)-permuted
    return dict(wmsg=wmsg, wself=wself, wpar=wpar, wk=wk, wfin=wfin)


def _pack_shard(bx, px, msgs16, maskbias, tiles):
    """Per-core arrays -> device layouts. Inputs already node-sharded+padded:
    bx/px (npad, 256) f32, msgs16 (npad, 16, 256) bf16, maskbias (npad, 18) f32."""
    npad = bx.shape[0]
    assert npad == tiles * 128

    def xpose(x):  # (npad, 256) -> (2, tiles, 128, 128) bf16 [dh, t, d, nl]
        x = x.astype(BF16).reshape(tiles, 128, 2, 128)
        return np.ascontiguousarray(x.transpose(2, 0, 3, 1))

    m = msgs16.reshape(tiles, 128, NMSG, 2, 128)  # [t, nl, s, dh, d]
    m = np.ascontiguousarray(m.transpose(3, 0, 4, 2, 1))  # [dh, t, d, s, nl]
    return dict(
        msgs=m.reshape(2, tiles, 128, NMSG * 128),
        bx=xpose(bx),
        px=xpose(px),
        bias=np.ascontiguousarray(maskbias.reshape(tiles, 128, NSLOT)),
    )


def _make_in_maps(batch_x, parallel_node_index, ccw_msg, ccw_mask, cw_msg,
                  cw_mask, weights, npc, ncores, tiles):
    n = npc * ncores
    npad = tiles * 128
    bx = np.asarray(batch_x, np.float32)
    idx = np.asarray(parallel_node_index).astype(np.int64)
    px = bx[idx]

    msgs = np.empty((n, NMSG, D), dtype=BF16)
    msgs[:, 0:8] = np.asarray(ccw_msg, np.float32)
    msgs[:, 8:16] = np.asarray(cw_msg, np.float32)

    mb = np.zeros((n, NSLOT), np.float32)
    mb[:, 0:8] = np.where(np.asarray(ccw_mask, bool), 0.0, BIG_NEG)
    mb[:, 10:18] = np.where(np.asarray(cw_mask, bool), 0.0, BIG_NEG)

    in_maps = []
    for c in range(ncores):
        sl = slice(c * npc, (c + 1) * npc)
        pad = npad - npc
        bxc = np.pad(bx[sl], ((0, pad), (0, 0)))
        pxc = np.pad(px[sl], ((0, pad), (0, 0)))
        mc = np.pad(msgs[sl], ((0, pad), (0, 0), (0, 0)))
        mbc = np.pad(mb[sl], ((0, pad), (0, 0)))
        m = _pack_shard(bxc, pxc, mc, mbc, tiles)
        m.update(weights)
        in_maps.append(m)
    return in_maps


# --------------------------------------------------------------------------
# Entry point
# --------------------------------------------------------------------------

def kernel(batch_x, parallel_node_index, ccw_msg, ccw_mask, cw_msg, cw_mask,
           W_self, W_parallel, W_q, W_k, W_v, W_final):
    from concourse.bass_utils import run_bass_kernel_spmd

    if "nc" not in _CACHE:
        _CACHE["nc"] = _build_program(TILES)
    nc = _CACHE["nc"]

    weights = _pack_weights(W_self, W_parallel, W_q, W_k, W_v, W_final)
    in_maps = _make_in_maps(batch_x, parallel_node_index, ccw_msg, ccw_mask,
                            cw_msg, cw_mask, weights, NPC, NCORES, TILES)

    trace = bool(_CACHE.get("trace", False))
    res = run_bass_kernel_spmd(nc, in_maps, core_ids=list(range(NCORES)),
                               trace=trace)
    _CACHE["last_result"] = res

    out = np.concatenate(
        [res.results[c]["out"].reshape(TILES * 128, D)[:NPC]
         for c in range(NCORES)], axis=0)
    return np.ascontiguousarray(out.astype(np.float32))
